# revision 9
# baseline (speedup 1.0000x reference)
"""Multi-head attention kernel for 8 Trainium2 NeuronCores.

Problem: embeddings [4, 2048, 1024], 16 heads x 64 dim, torch nn.Linear
convention (x @ W.T + b) for Q/K/V/O projections.

Sharding: batch (4) x head-halves (2) -> 8 cores. Core c handles batch
c//2, local heads (c%2)*8..(c%2)*8+8. Output projection is row-sharded;
host sums the two partial outputs per batch element and adds an
effective bias bo_eff = bo + wo @ bv (the V bias commutes through the
probability-weighted sum exactly, since probs sum to 1). The K bias is
dropped entirely: softmax over keys is invariant to per-query constants
and bk contributes only q.bk + bq.bk terms, which are constant in k.

Per-core dataflow, pair-major (pair = 2 heads sharing the PE array):
  for pair p (4): for q-block qb (4): for key-tile kt (16): one step.
Scores land in a 3-slot PSUM ring ([128, 3072] f32, 6 banks); exp runs
on ScalarE batched 2 steps per ACTIVATE ([128, 2048], split in two when
the ring wraps). U (exp @ V) accumulates in 1 PSUM bank per block;
sumexp partials are 4 col-tiled M=1 ones-matmuls into a transient bank,
accumulated across windows on the DVE into SBUF. Projections / output
projections run as filler chains (8 accumulating matmuls back-to-back,
weight loads hidden) interleaved into the attention stream.
"""

import sys

sys.path.insert(0, "/opt/trn_rl_repo")

import numpy as np
import ml_dtypes

import concourse.bass as bass
import concourse.bacc as bacc
import concourse.mybir as mybir
import concourse.tile as tile
from concourse.bass_utils import run_bass_kernel_spmd

BF16 = mybir.dt.bfloat16
F32 = mybir.dt.float32
NPBF16 = ml_dtypes.bfloat16

B, S, E = 4, 2048, 1024
D = 64             # head dim
OL = 512           # local output dim (8 heads)
NP = 4             # head pairs per core
N_CORES = 8
QB = 512           # q-block
NQB = 4
NKT = 16           # key tiles of 128
NET = 8            # embed tiles of 128
NSTEP = 256        # NP * NQB * NKT
NWIN = 128


def build_program():
    from contextlib import ExitStack

    nc = bacc.Bacc("TRN2", debug=False, num_devices=N_CORES)

    xp = nc.dram_tensor("xp", [128, 4 * 4096], BF16, kind="ExternalInput")
    wqp = nc.dram_tensor("wqp", [128, 4096], BF16, kind="ExternalInput")
    wkp = nc.dram_tensor("wkp", [128, 4096], BF16, kind="ExternalInput")
    wvp = nc.dram_tensor("wvp", [128, 4096], BF16, kind="ExternalInput")
    wop = nc.dram_tensor("wop", [128, 4096], BF16, kind="ExternalInput")
    bqc = nc.dram_tensor("bqc", [128, 4], F32, kind="ExternalInput")
    yT = nc.dram_tensor("yT", [E, S], F32, kind="ExternalOutput")

    with tile.TileContext(nc) as tc, ExitStack() as est:
        xsb_p = est.enter_context(tc.tile_pool(name="xsb", bufs=4))
        wq_p = est.enter_context(tc.tile_pool(name="wq", bufs=4))
        wk_p = est.enter_context(tc.tile_pool(name="wk", bufs=4))
        wv_p = est.enter_context(tc.tile_pool(name="wv", bufs=1))
        wo_p = est.enter_context(tc.tile_pool(name="wo", bufs=4))
        bias_p = est.enter_context(tc.tile_pool(name="bias", bufs=4))
        qt_p = est.enter_context(tc.tile_pool(name="qt", bufs=4))
        kt_p = est.enter_context(tc.tile_pool(name="kt", bufs=4))
        vb_p = est.enter_context(tc.tile_pool(name="vb", bufs=NKT))
        at_p = est.enter_context(tc.tile_pool(name="at", bufs=16))
        et_p = est.enter_context(tc.tile_pool(name="et", bufs=3))
        se_p = est.enter_context(tc.tile_pool(name="sea", bufs=2))
        usb_p = est.enter_context(tc.tile_pool(name="usb", bufs=2))
        nrm_p = est.enter_context(tc.tile_pool(name="nrm", bufs=4))
        ys_p = est.enter_context(tc.tile_pool(name="ys", bufs=4))
        ring_p = est.enter_context(tc.tile_pool(name="ring", bufs=1,
                                                space="PSUM"))
        u_p = est.enter_context(tc.tile_pool(name="u", bufs=1, space="PSUM"))
        pj_p = est.enter_context(tc.tile_pool(name="pj", bufs=1, space="PSUM"))

        # ---- ACT table warm-up while DMAs start ----
        warm = bias_p.tile([1, 16], F32, tag="warm", name="warm")
        nc.vector.memset(warm[:], 0.0)
        warm2 = bias_p.tile([1, 16], F32, tag="warm2", name="warm2")
        nc.scalar.activation(warm2[:], warm[:],
                             mybir.ActivationFunctionType.Exp)

        # ---- persistent SBUF tiles ----
        xsb = [xsb_p.tile([128, 4096], BF16, tag="xsb", name="xsb")
               for _ in range(4)]
        wqsb = [wq_p.tile([128, 1024], BF16, tag="wq", name="wqsb")
                for _ in range(4)]
        wksb = [wk_p.tile([128, 1024], BF16, tag="wk", name="wksb")
                for _ in range(4)]
        wvsb = wv_p.tile([128, 4096], BF16, tag="wv", name="wvsb")
        wosb = [wo_p.tile([128, 1024], BF16, tag="wo", name="wosb")
                for _ in range(4)]
        bqs = bias_p.tile([128, 4], F32, tag="bqc", name="bqs")
        ones32 = bias_p.tile([128, 32], BF16, tag="ones32", name="ones32")
        nc.vector.memset(ones32[:], 1.0)

        qts = [qt_p.tile([128, S], BF16, tag="qt", name="qts")
               for _ in range(NP)]
        kts = [kt_p.tile([128, S], BF16, tag="kt", name="kts")
               for _ in range(NP)]
        vbs = [vb_p.tile([128, OL], BF16, tag="vb", name="vbs")
               for _ in range(NKT)]
        atts = [[at_p.tile([128, QB], BF16, tag="at", name="atts")
                 for _ in range(NP)] for _ in range(NQB)]

        # ---- input DMAs, priority order ----
        nc.sync.dma_start(xsb[0][:], xp[:, 0:4096])
        nc.sync.dma_start(wqsb[0][:], wqp[:, 0:1024])
        nc.sync.dma_start(wksb[0][:], wkp[:, 0:1024])
        nc.sync.dma_start(bqs[:], bqc[:])
        nc.sync.dma_start(wvsb[:], wvp[:])
        for j in range(1, 4):
            nc.sync.dma_start(xsb[j][:], xp[:, j * 4096:(j + 1) * 4096])
        for o in range(1, 4):
            nc.sync.dma_start(wksb[o][:], wkp[:, o * 1024:(o + 1) * 1024])
            nc.sync.dma_start(wqsb[o][:], wqp[:, o * 1024:(o + 1) * 1024])
        for p2 in range(4):
            nc.sync.dma_start(wosb[p2][:], wop[:, p2 * 1024:(p2 + 1) * 1024])

        # ---- PSUM ring (3 slots x [128,1024]) ----
        ring = ring_p.tile([128, 3072], F32, tag="ring", name="ring")

        # ---- emitters ----
        def qk_group(p, j, which):
            w = wqsb[p] if which == "q" else wksb[p]
            dest = qts[p] if which == "q" else kts[p]
            acc = pj_p.tile([128, QB], F32, tag="pj", name="pj")
            for e in range(NET):
                nc.tensor.matmul(
                    acc[:],
                    w[:, e * 128:(e + 1) * 128],
                    xsb[j][:, e * 512:(e + 1) * 512],
                    start=(e == 0), stop=(e == NET - 1),
                )
            if which == "q":
                nc.vector.tensor_scalar_add(
                    dest[:, j * QB:(j + 1) * QB], acc[:], bqs[:, p:p + 1])
            else:
                nc.vector.tensor_copy(dest[:, j * QB:(j + 1) * QB], acc[:])

        def v_group(t):
            jj, tt = t // 4, t % 4
            acc = pj_p.tile([128, OL], F32, tag="pj", name="pjv")
            for e in range(NET):
                nc.tensor.matmul(
                    acc[:],
                    xsb[jj][:, e * 512 + tt * 128: e * 512 + (tt + 1) * 128],
                    wvsb[:, e * 512:(e + 1) * 512],
                    start=(e == 0), stop=(e == NET - 1),
                )
            nc.vector.tensor_copy(vbs[t][:], acc[:])

        def og_group(qb, eo):
            y = pj_p.tile([128, QB], F32, tag="pj", name="pjo")
            for p2 in range(4):
                nc.tensor.matmul(
                    y[:],
                    wosb[p2][:, eo * 128:(eo + 1) * 128],
                    atts[qb][p2][:],
                    start=(p2 == 0), stop=(p2 == 3),
                )
            ysb = ys_p.tile([128, QB], F32, tag="ys", name="ysb")
            nc.vector.tensor_copy(ysb[:], y[:])
            nc.sync.dma_start(
                yT[eo * 128:(eo + 1) * 128, qb * QB:(qb + 1) * QB], ysb[:])

        def emit_scores(s):
            p, qb, kt = s // 64, (s % 64) // 16, s % 16
            off = (s % 3) * 1024
            for h2 in range(2):
                nc.tensor.matmul(
                    ring[:, off + h2 * QB: off + (h2 + 1) * QB],
                    kts[p][h2 * 64:(h2 + 1) * 64, kt * 128:(kt + 1) * 128],
                    qts[p][h2 * 64:(h2 + 1) * 64, qb * QB:(qb + 1) * QB],
                    start=True, stop=True, tile_position=(h2 * 64, 0),
                    skip_group_check=True,
                )

        # ---- filler schedule (window -> list of thunks) ----
        sched = {}

        def put(w, th):
            sched.setdefault(w, []).append(th)

        def qg(p, j):
            return lambda: qk_group(p, j, "q")

        def kg(p, j):
            return lambda: qk_group(p, j, "k")

        for t in range(4, 16):               # V(t) needed at step t
            put((t - 4) // 2, lambda t=t: v_group(t))
        put(0, kg(0, 1))
        put(2, kg(0, 2))
        put(4, kg(0, 3))
        put(6, qg(0, 1))
        put(10, qg(0, 2))
        put(14, qg(0, 3))
        for p in range(1, NP):
            base = {1: 16, 2: 40, 3: 66}[p]
            gap = {1: 2, 2: 3, 3: 4}[p]
            items = [kg(p, 0), kg(p, 1), kg(p, 2), kg(p, 3),
                     qg(p, 0), qg(p, 1), qg(p, 2), qg(p, 3)]
            for idx, th in enumerate(items):
                put(base + gap * idx, th)
        for qb in range(3):                  # og(qb) after pair3 block qb
            for eo in range(NET):
                put(104 + 8 * qb + eo, lambda qb=qb, eo=eo: og_group(qb, eo))

        # ---- prologue ----
        qk_group(0, 0, "q")
        qk_group(0, 0, "k")
        for t in range(4):
            v_group(t)
        emit_scores(0)
        emit_scores(1)

        # ---- main loop ----
        cur = {}
        for w in range(NWIN):
            s0, s1 = 2 * w, 2 * w + 1
            p, qb = s0 // 64, (s0 % 64) // 16
            kt0, kt1 = s0 % 16, s1 % 16
            a, b = s0 % 3, s1 % 3

            # exp batch for (s0, s1)
            et = et_p.tile([128, 2048], BF16, tag="et", name="et")
            if b == a + 1:
                nc.scalar.activation(
                    et[:], ring[:, a * 1024: a * 1024 + 2048],
                    mybir.ActivationFunctionType.Exp, scale=0.125)
            else:  # wrap (a=2, b=0)
                nc.scalar.activation(
                    et[:, 0:1024], ring[:, 2048:3072],
                    mybir.ActivationFunctionType.Exp, scale=0.125)
                nc.scalar.activation(
                    et[:, 1024:2048], ring[:, 0:1024],
                    mybir.ActivationFunctionType.Exp, scale=0.125)

            if kt0 == 0:
                cur["u"] = u_p.tile([128, QB], F32, tag="u", name="u")
                cur["sea"] = se_p.tile([128, QB], F32, tag="sea", name="sea")

            # fillers
            for th in sched.get(w, []):
                th()

            # scores lookahead
            if s0 + 2 < NSTEP:
                emit_scores(s0 + 2)

            # U matmuls for s0, s1 (chained, same accumulation group)
            u = cur["u"]
            for si, kt in ((0, kt0), (1, kt1)):
                for h2 in range(2):
                    hl = 2 * p + h2
                    nc.tensor.matmul(
                        u[h2 * 64:(h2 + 1) * 64, :],
                        vbs[kt][:, hl * D:(hl + 1) * D],
                        et[:, si * 1024 + h2 * QB: si * 1024 + (h2 + 1) * QB],
                        start=(kt == 0), stop=(kt == NKT - 1),
                        tile_position=(0, h2 * 64),
                        skip_group_check=True,
                    )

            # sumexp partials: 4 col-tiled M=1 matmuls, transient bank
            seb = pj_p.tile([128, QB], F32, tag="pj", name="pjs")
            for g in range(4):
                q2, h2 = g // 2, g % 2
                nc.tensor.matmul(
                    seb[g * 32:(g + 1) * 32, :],
                    ones32[:],
                    et[:, q2 * 1024 + h2 * QB: q2 * 1024 + (h2 + 1) * QB],
                    start=True, stop=True,
                    tile_position=(0, g * 32),
                    skip_group_check=True,
                )
            sea = cur["sea"]
            if kt0 == 0:
                nc.vector.tensor_copy(sea[:], seb[:])
            else:
                nc.vector.tensor_add(sea[:], sea[:], seb[:])

            # scores lookahead (second; gated on this window's exp)
            if s0 + 3 < NSTEP:
                emit_scores(s0 + 3)

            # block end: evacuate U, normalize into atts
            if kt1 == NKT - 1:
                ub = usb_p.tile([128, QB], BF16, tag="usb", name="ub")
                nc.vector.tensor_copy(ub[:], u[:])
                for h2 in range(2):
                    sa = nrm_p.tile([1, QB], F32, tag="sa", name="sa")
                    nc.vector.tensor_copy(
                        sa[:], sea[h2 * 32:h2 * 32 + 1, :])
                    sb2 = nrm_p.tile([1, QB], F32, tag="sb2", name="sb2")
                    nc.vector.tensor_copy(
                        sb2[:], sea[64 + h2 * 32:64 + h2 * 32 + 1, :])
                    rcs = nrm_p.tile([1, QB], F32, tag="rcs", name="rcs")
                    nc.vector.tensor_add(rcs[:], sa[:], sb2[:])
                    rcr = nrm_p.tile([1, QB], F32, tag="rcr", name="rcr")
                    nc.vector.reciprocal_approx_fast(rcr[:], rcs[:])
                    bcf = nrm_p.tile([128, QB], F32, tag="bcf", name="bcf")
                    nc.gpsimd.partition_broadcast(bcf[:], rcr[:])
                    nc.vector.tensor_mul(
                        atts[qb][p][h2 * 64:(h2 + 1) * 64, :],
                        ub[h2 * 64:(h2 + 1) * 64, :],
                        bcf[h2 * 64:(h2 + 1) * 64, :])

        # ---- tail: last q-block's output projection ----
        for eo in range(NET):
            og_group(3, eo)

    nc.compile()
    return nc


_CACHED = {}


def _get_program():
    if "nc" not in _CACHED:
        _CACHED["nc"] = build_program()
    return _CACHED["nc"]


def make_inputs(embeddings, wq, bq, wk, bk, wv, bv, wo, bo):
    """Host-side sharding: per-core input maps."""
    in_maps = []
    for c in range(N_CORES):
        b, half = c // 2, c % 2
        sl = slice(half * OL, (half + 1) * OL)
        # xp[p, j*4096 + e*512 + u] = embeddings[b, j*512+u, e*128+p]
        xpk = np.ascontiguousarray(
            embeddings[b].reshape(4, 512, 8, 128).transpose(3, 0, 2, 1)
            .reshape(128, 16384)).astype(NPBF16)
        # wqp[p, o*1024 + e*128 + c2] = wq[sl][o*128+c2, e*128+p]
        wqk = np.ascontiguousarray(
            wq[sl, :].reshape(4, 128, 8, 128).transpose(3, 0, 2, 1)
            .reshape(128, 4096)).astype(NPBF16)
        wkk = np.ascontiguousarray(
            wk[sl, :].reshape(4, 128, 8, 128).transpose(3, 0, 2, 1)
            .reshape(128, 4096)).astype(NPBF16)
        # wvp[p, e*512 + f] = wv[sl][f, e*128+p]
        wvk = np.ascontiguousarray(
            wv[sl, :].reshape(512, 8, 128).transpose(2, 1, 0)
            .reshape(128, 4096)).astype(NPBF16)
        # wop[p, p2*1024 + c2] = wo[c2, sl][p2*128+p]
        wok = np.ascontiguousarray(
            wo[:, sl].T.reshape(4, 128, 1024).transpose(1, 0, 2)
            .reshape(128, 4096)).astype(NPBF16)
        in_maps.append({
            "xp": xpk,
            "wqp": wqk,
            "wkp": wkk,
            "wvp": wvk,
            "wop": wok,
            "bqc": np.ascontiguousarray(
                bq[sl].reshape(4, 128).T).astype(np.float32),
        })
    return in_maps


def unshard(results, wo, bv, bo):
    bo_eff = bo + wo @ bv
    out = np.empty((B, S, E), np.float32)
    for b in range(B):
        yt = results[2 * b]["yT"] + results[2 * b + 1]["yT"]
        out[b] = yt.T + bo_eff[None, :]
    return out


def kernel(embeddings, wq, bq, wk, bk, wv, bv, wo, bo, _trace=False):
    embeddings = np.asarray(embeddings, np.float32)
    wq, bq = np.asarray(wq, np.float32), np.asarray(bq, np.float32)
    wk, bk = np.asarray(wk, np.float32), np.asarray(bk, np.float32)
    wv, bv = np.asarray(wv, np.float32), np.asarray(bv, np.float32)
    wo, bo = np.asarray(wo, np.float32), np.asarray(bo, np.float32)
    nc = _get_program()
    in_maps = make_inputs(embeddings, wq, bq, wk, bk, wv, bv, wo, bo)
    res = run_bass_kernel_spmd(
        nc, in_maps, core_ids=list(range(N_CORES)), trace=_trace)
    out = unshard(res.results, wo, bv, bo)
    if _trace:
        kernel.last_result = res
    return out


# revision 14
# speedup vs baseline: 1.1120x; 1.1120x over previous
"""Multi-head attention kernel for 8 Trainium2 NeuronCores.

Problem: embeddings [4, 2048, 1024], 16 heads x 64 dim, torch nn.Linear
convention (x @ W.T + b) for Q/K/V/O projections.

Sharding: batch (4) x head-halves (2) -> 8 cores. Core c handles batch
c//2, local heads (c%2)*8..(c%2)*8+8. Output projection is row-sharded;
host sums the two partial outputs per batch element and adds an
effective bias bo_eff = bo + wo @ bv (the V bias commutes through the
probability-weighted sum exactly, since probs sum to 1). The K bias is
dropped entirely: softmax over keys is invariant to per-query constants
and bk contributes only q.bk + bq.bk terms, which are constant in k.

Per-core dataflow, pair-major (pair = 2 heads sharing the PE array):
  for pair p (4): for q-block qb (4): for key-tile kt (16): one step.
Scores land in a 3-slot PSUM ring ([128, 3072] f32, 6 banks); exp runs
on ScalarE batched 2 steps per ACTIVATE ([128, 2048], split in two when
the ring wraps). U (exp @ V) accumulates in 1 PSUM bank per block;
sumexp partials are 4 col-tiled M=1 ones-matmuls into a transient bank,
accumulated across windows on the DVE into SBUF. Projections / output
projections run as filler chains (8 accumulating matmuls back-to-back,
weight loads hidden) interleaved into the attention stream.
"""

import sys

sys.path.insert(0, "/opt/trn_rl_repo")

import numpy as np
import ml_dtypes

import concourse.bass as bass
import concourse.bacc as bacc
import concourse.mybir as mybir
import concourse.tile as tile
from concourse.bass_utils import run_bass_kernel_spmd

BF16 = mybir.dt.bfloat16
F32 = mybir.dt.float32
NPBF16 = ml_dtypes.bfloat16

B, S, E = 4, 2048, 1024
D = 64             # head dim
OL = 512           # local output dim (8 heads)
NP = 4             # head pairs per core
N_CORES = 8
QB = 512           # q-block
NQB = 4
NKT = 16           # key tiles of 128
NET = 8            # embed tiles of 128
NSTEP = 256        # NP * NQB * NKT
NWIN = 128


def build_program():
    from contextlib import ExitStack

    nc = bacc.Bacc("TRN2", debug=False, num_devices=N_CORES)

    xp = nc.dram_tensor("xp", [128, 4 * 4096], BF16, kind="ExternalInput")
    wqp = nc.dram_tensor("wqp", [128, 4096], BF16, kind="ExternalInput")
    wkp = nc.dram_tensor("wkp", [128, 4096], BF16, kind="ExternalInput")
    wvp = nc.dram_tensor("wvp", [128, 4096], BF16, kind="ExternalInput")
    wop = nc.dram_tensor("wop", [128, 4096], BF16, kind="ExternalInput")
    bqc = nc.dram_tensor("bqc", [128, 4], F32, kind="ExternalInput")
    yT = nc.dram_tensor("yT", [E, S], F32, kind="ExternalOutput")

    with tile.TileContext(nc) as tc, ExitStack() as est:
        xsb_p = est.enter_context(tc.tile_pool(name="xsb", bufs=4))
        wq_p = est.enter_context(tc.tile_pool(name="wq", bufs=4))
        wk_p = est.enter_context(tc.tile_pool(name="wk", bufs=4))
        wv_p = est.enter_context(tc.tile_pool(name="wv", bufs=1))
        wo_p = est.enter_context(tc.tile_pool(name="wo", bufs=4))
        bias_p = est.enter_context(tc.tile_pool(name="bias", bufs=4))
        qt_p = est.enter_context(tc.tile_pool(name="qt", bufs=4))
        kt_p = est.enter_context(tc.tile_pool(name="kt", bufs=4))
        vb_p = est.enter_context(tc.tile_pool(name="vb", bufs=NKT))
        at_p = est.enter_context(tc.tile_pool(name="at", bufs=16))
        et_p = est.enter_context(tc.tile_pool(name="et", bufs=4))
        usb_p = est.enter_context(tc.tile_pool(name="usb", bufs=2))
        nrm_p = est.enter_context(tc.tile_pool(name="nrm", bufs=4))
        ys_p = est.enter_context(tc.tile_pool(name="ys", bufs=4))
        ring_p = est.enter_context(tc.tile_pool(name="ring", bufs=1,
                                                space="PSUM"))
        u_p = est.enter_context(tc.tile_pool(name="u", bufs=1, space="PSUM"))
        se_psum_p = est.enter_context(tc.tile_pool(name="sep", bufs=1,
                                                   space="PSUM"))
        pj_p = est.enter_context(tc.tile_pool(name="pj", bufs=2, space="PSUM"))

        # ---- ACT table warm-up while DMAs start ----
        warm = bias_p.tile([1, 16], F32, tag="warm", name="warm")
        nc.vector.memset(warm[:], 0.0)
        warm2 = bias_p.tile([1, 16], F32, tag="warm2", name="warm2")
        nc.scalar.activation(warm2[:], warm[:],
                             mybir.ActivationFunctionType.Exp)

        # ---- persistent SBUF tiles ----
        xsb = [xsb_p.tile([128, 4096], BF16, tag="xsb", name="xsb")
               for _ in range(4)]
        wqsb = [wq_p.tile([128, 1024], BF16, tag="wq", name="wqsb")
                for _ in range(4)]
        wksb = [wk_p.tile([128, 1024], BF16, tag="wk", name="wksb")
                for _ in range(4)]
        wvsb = wv_p.tile([128, 4096], BF16, tag="wv", name="wvsb")
        wosb = [wo_p.tile([128, 1024], BF16, tag="wo", name="wosb")
                for _ in range(4)]
        bqs = bias_p.tile([128, 4], F32, tag="bqc", name="bqs")
        ones32 = bias_p.tile([128, 32], BF16, tag="ones32", name="ones32")
        nc.vector.memset(ones32[:], 1.0)

        qts = [qt_p.tile([128, S], BF16, tag="qt", name="qts")
               for _ in range(NP)]
        kts = [kt_p.tile([128, S], BF16, tag="kt", name="kts")
               for _ in range(NP)]
        vbs = [vb_p.tile([128, OL], BF16, tag="vb", name="vbs")
               for _ in range(NKT)]
        atts = [[at_p.tile([128, QB], BF16, tag="at", name="atts")
                 for _ in range(NP)] for _ in range(NQB)]

        # ---- input DMAs, priority order ----
        nc.sync.dma_start(xsb[0][:], xp[:, 0:4096])
        nc.sync.dma_start(wqsb[0][:], wqp[:, 0:1024])
        nc.sync.dma_start(wksb[0][:], wkp[:, 0:1024])
        nc.sync.dma_start(bqs[:], bqc[:])
        nc.sync.dma_start(wvsb[:], wvp[:])
        for j in range(1, 4):
            nc.sync.dma_start(xsb[j][:], xp[:, j * 4096:(j + 1) * 4096])
        for o in range(1, 4):
            nc.sync.dma_start(wksb[o][:], wkp[:, o * 1024:(o + 1) * 1024])
            nc.sync.dma_start(wqsb[o][:], wqp[:, o * 1024:(o + 1) * 1024])
        for p2 in range(4):
            nc.sync.dma_start(wosb[p2][:], wop[:, p2 * 1024:(p2 + 1) * 1024])

        # ---- PSUM ring (2 slots x [128,1024]) ----
        ring = ring_p.tile([128, 2048], F32, tag="ring", name="ring")

        # ---- emitters ----
        def qk_group(p, j, which):
            w = wqsb[p] if which == "q" else wksb[p]
            dest = qts[p] if which == "q" else kts[p]
            acc = pj_p.tile([128, QB], F32, tag="pj", name="pj")
            for e in range(NET):
                nc.tensor.matmul(
                    acc[:],
                    w[:, e * 128:(e + 1) * 128],
                    xsb[j][:, e * 512:(e + 1) * 512],
                    start=(e == 0), stop=(e == NET - 1),
                )
            if which == "q":
                nc.vector.tensor_scalar_add(
                    dest[:, j * QB:(j + 1) * QB], acc[:], bqs[:, p:p + 1])
            else:
                nc.vector.tensor_copy(dest[:, j * QB:(j + 1) * QB], acc[:])

        def v_group(t):
            jj, tt = t // 4, t % 4
            acc = pj_p.tile([128, OL], F32, tag="pj", name="pjv")
            for e in range(NET):
                nc.tensor.matmul(
                    acc[:],
                    xsb[jj][:, e * 512 + tt * 128: e * 512 + (tt + 1) * 128],
                    wvsb[:, e * 512:(e + 1) * 512],
                    start=(e == 0), stop=(e == NET - 1),
                )
            nc.vector.tensor_copy(vbs[t][:], acc[:])

        def og_group(qb, eo):
            y = pj_p.tile([128, QB], F32, tag="pj", name="pjo")
            for p2 in range(4):
                nc.tensor.matmul(
                    y[:],
                    wosb[p2][:, eo * 128:(eo + 1) * 128],
                    atts[qb][p2][:],
                    start=(p2 == 0), stop=(p2 == 3),
                )
            ysb = ys_p.tile([128, QB], F32, tag="ys", name="ysb")
            nc.vector.tensor_copy(ysb[:], y[:])
            nc.sync.dma_start(
                yT[eo * 128:(eo + 1) * 128, qb * QB:(qb + 1) * QB], ysb[:])

        def emit_scores(s):
            p, qb, kt = s // 64, (s % 64) // 16, s % 16
            off = (s % 2) * 1024
            for h2 in range(2):
                nc.tensor.matmul(
                    ring[:, off + h2 * QB: off + (h2 + 1) * QB],
                    kts[p][h2 * 64:(h2 + 1) * 64, kt * 128:(kt + 1) * 128],
                    qts[p][h2 * 64:(h2 + 1) * 64, qb * QB:(qb + 1) * QB],
                    start=True, stop=True, tile_position=(h2 * 64, 0),
                    skip_group_check=True,
                )

        # ---- filler schedule (window -> list of thunks) ----
        sched = {}

        def put(w, th):
            sched.setdefault(w, []).append(th)

        def qg(p, j):
            return lambda: qk_group(p, j, "q")

        def kg(p, j):
            return lambda: qk_group(p, j, "k")

        for t in range(4, 16):               # V(t) needed at step t
            put((t - 4) // 2, lambda t=t: v_group(t))
        put(0, kg(0, 1))
        put(2, kg(0, 2))
        put(4, kg(0, 3))
        put(6, qg(0, 1))
        put(10, qg(0, 2))
        put(14, qg(0, 3))
        for p in range(1, NP):
            base = {1: 16, 2: 40, 3: 66}[p]
            gap = {1: 2, 2: 3, 3: 4}[p]
            items = [kg(p, 0), kg(p, 1), kg(p, 2), kg(p, 3),
                     qg(p, 0), qg(p, 1), qg(p, 2), qg(p, 3)]
            for idx, th in enumerate(items):
                put(base + gap * idx, th)
        for qb in range(3):                  # og(qb) after pair3 block qb
            for eo in range(NET):
                put(104 + 8 * qb + eo, lambda qb=qb, eo=eo: og_group(qb, eo))

        # ---- prologue ----
        qk_group(0, 0, "q")
        qk_group(0, 0, "k")
        for t in range(4):
            v_group(t)
        emit_scores(0)
        emit_scores(1)

        # ---- main loop ----
        cur = {}
        for w in range(NWIN):
            s0, s1 = 2 * w, 2 * w + 1
            p, qb = s0 // 64, (s0 % 64) // 16
            kt0, kt1 = s0 % 16, s1 % 16

            # exp for s0, s1 (separate so scores(s0+2) can overlap exp(s1))
            et0 = et_p.tile([128, 1024], BF16, tag="et", name="et0")
            nc.scalar.activation(
                et0[:], ring[:, 0:1024],
                mybir.ActivationFunctionType.Exp, scale=0.125)
            et1 = et_p.tile([128, 1024], BF16, tag="et", name="et1")
            nc.scalar.activation(
                et1[:], ring[:, 1024:2048],
                mybir.ActivationFunctionType.Exp, scale=0.125)

            if kt0 == 0:
                cur["u"] = u_p.tile([128, QB], F32, tag="u", name="u")
                cur["se"] = se_psum_p.tile([128, QB], F32, tag="se",
                                           name="se")

            # fillers
            for th in sched.get(w, []):
                th()

            # scores lookahead (slot 0; gated on exp(s0) only)
            if s0 + 2 < NSTEP:
                emit_scores(s0 + 2)

            # U matmuls for s0, s1 (chained, same accumulation group)
            u = cur["u"]
            for et, kt in ((et0, kt0), (et1, kt1)):
                for h2 in range(2):
                    hl = 2 * p + h2
                    nc.tensor.matmul(
                        u[h2 * 64:(h2 + 1) * 64, :],
                        vbs[kt][:, hl * D:(hl + 1) * D],
                        et[:, h2 * QB:(h2 + 1) * QB],
                        start=(kt == 0), stop=(kt == NKT - 1),
                        tile_position=(0, h2 * 64),
                        skip_group_check=True,
                    )

            # sumexp: 4 col-tiled M=32 matmuls accumulating over the block
            se = cur["se"]
            for g in range(4):
                q2, h2 = g // 2, g % 2
                nc.tensor.matmul(
                    se[g * 32:(g + 1) * 32, :],
                    ones32[:],
                    (et0 if q2 == 0 else et1)[:, h2 * QB:(h2 + 1) * QB],
                    start=(kt0 == 0), stop=(kt1 == NKT - 1),
                    tile_position=(0, g * 32),
                    skip_group_check=True,
                )

            # scores lookahead (slot 1; gated on exp(s1))
            if s0 + 3 < NSTEP:
                emit_scores(s0 + 3)

            # block end: evacuate U, normalize into atts
            if kt1 == NKT - 1:
                ub = usb_p.tile([128, QB], BF16, tag="usb", name="ub")
                nc.vector.tensor_copy(ub[:], u[:])
                for h2 in range(2):
                    sa = nrm_p.tile([1, QB], F32, tag="sa", name="sa")
                    nc.vector.tensor_copy(
                        sa[:], se[h2 * 32:h2 * 32 + 1, :])
                    sb2 = nrm_p.tile([1, QB], F32, tag="sb2", name="sb2")
                    nc.vector.tensor_copy(
                        sb2[:], se[64 + h2 * 32:64 + h2 * 32 + 1, :])
                    rcs = nrm_p.tile([1, QB], F32, tag="rcs", name="rcs")
                    nc.vector.tensor_add(rcs[:], sa[:], sb2[:])
                    rcr = nrm_p.tile([1, QB], F32, tag="rcr", name="rcr")
                    nc.vector.reciprocal_approx_fast(rcr[:], rcs[:])
                    bcf = nrm_p.tile([128, QB], F32, tag="bcf", name="bcf")
                    nc.gpsimd.partition_broadcast(bcf[:], rcr[:])
                    nc.vector.tensor_mul(
                        atts[qb][p][h2 * 64:(h2 + 1) * 64, :],
                        ub[h2 * 64:(h2 + 1) * 64, :],
                        bcf[h2 * 64:(h2 + 1) * 64, :])

        # ---- tail: last q-block's output projection ----
        for eo in range(NET):
            og_group(3, eo)

    nc.compile()
    return nc


_CACHED = {}


def _get_program():
    if "nc" not in _CACHED:
        _CACHED["nc"] = build_program()
    return _CACHED["nc"]


def make_inputs(embeddings, wq, bq, wk, bk, wv, bv, wo, bo):
    """Host-side sharding: per-core input maps."""
    in_maps = []
    for c in range(N_CORES):
        b, half = c // 2, c % 2
        sl = slice(half * OL, (half + 1) * OL)
        # xp[p, j*4096 + e*512 + u] = embeddings[b, j*512+u, e*128+p]
        xpk = np.ascontiguousarray(
            embeddings[b].reshape(4, 512, 8, 128).transpose(3, 0, 2, 1)
            .reshape(128, 16384)).astype(NPBF16)
        # wqp[p, o*1024 + e*128 + c2] = wq[sl][o*128+c2, e*128+p]
        wqk = np.ascontiguousarray(
            wq[sl, :].reshape(4, 128, 8, 128).transpose(3, 0, 2, 1)
            .reshape(128, 4096)).astype(NPBF16)
        wkk = np.ascontiguousarray(
            wk[sl, :].reshape(4, 128, 8, 128).transpose(3, 0, 2, 1)
            .reshape(128, 4096)).astype(NPBF16)
        # wvp[p, e*512 + f] = wv[sl][f, e*128+p]
        wvk = np.ascontiguousarray(
            wv[sl, :].reshape(512, 8, 128).transpose(2, 1, 0)
            .reshape(128, 4096)).astype(NPBF16)
        # wop[p, p2*1024 + c2] = wo[c2, sl][p2*128+p]
        wok = np.ascontiguousarray(
            wo[:, sl].T.reshape(4, 128, 1024).transpose(1, 0, 2)
            .reshape(128, 4096)).astype(NPBF16)
        in_maps.append({
            "xp": xpk,
            "wqp": wqk,
            "wkp": wkk,
            "wvp": wvk,
            "wop": wok,
            "bqc": np.ascontiguousarray(
                bq[sl].reshape(4, 128).T).astype(np.float32),
        })
    return in_maps


def unshard(results, wo, bv, bo):
    bo_eff = bo + wo @ bv
    out = np.empty((B, S, E), np.float32)
    for b in range(B):
        yt = results[2 * b]["yT"] + results[2 * b + 1]["yT"]
        out[b] = yt.T + bo_eff[None, :]
    return out


def kernel(embeddings, wq, bq, wk, bk, wv, bv, wo, bo, _trace=False):
    embeddings = np.asarray(embeddings, np.float32)
    wq, bq = np.asarray(wq, np.float32), np.asarray(bq, np.float32)
    wk, bk = np.asarray(wk, np.float32), np.asarray(bk, np.float32)
    wv, bv = np.asarray(wv, np.float32), np.asarray(bv, np.float32)
    wo, bo = np.asarray(wo, np.float32), np.asarray(bo, np.float32)
    nc = _get_program()
    in_maps = make_inputs(embeddings, wq, bq, wk, bk, wv, bv, wo, bo)
    res = run_bass_kernel_spmd(
        nc, in_maps, core_ids=list(range(N_CORES)), trace=_trace)
    out = unshard(res.results, wo, bv, bo)
    if _trace:
        kernel.last_result = res
    return out


# revision 18
# speedup vs baseline: 1.6521x; 1.4856x over previous
"""Multi-head attention kernel for 8 Trainium2 NeuronCores.

Problem: embeddings [4, 2048, 1024], 16 heads x 64 dim, torch nn.Linear
convention (x @ W.T + b) for Q/K/V/O projections.

Sharding: batch (4) x head-halves (2) -> 8 cores. Core c handles batch
c//2, local heads (c%2)*8..(c%2)*8+8. Output projection is row-sharded;
host sums the two partial outputs per batch element and adds an
effective bias bo_eff = bo + wo @ bv (the V bias commutes through the
probability-weighted sum exactly, since probs sum to 1). The K bias is
dropped entirely: softmax over keys is invariant to per-query constants
and bk contributes only q.bk + bq.bk terms, which are constant in k.

Per-core dataflow, pair-major (pair = 2 heads sharing the PE array):
  for pair p (4): for q-block qb (4): for key-tile kt (16): one step.
Scores land in a 3-slot PSUM ring ([128, 3072] f32, 6 banks); exp runs
on ScalarE batched 2 steps per ACTIVATE ([128, 2048], split in two when
the ring wraps). U (exp @ V) accumulates in 1 PSUM bank per block;
sumexp partials are 4 col-tiled M=1 ones-matmuls into a transient bank,
accumulated across windows on the DVE into SBUF. Projections / output
projections run as filler chains (8 accumulating matmuls back-to-back,
weight loads hidden) interleaved into the attention stream.
"""

import sys

sys.path.insert(0, "/opt/trn_rl_repo")

import numpy as np
import ml_dtypes

import concourse.bass as bass
import concourse.bacc as bacc
import concourse.mybir as mybir
import concourse.tile as tile
from concourse.bass_utils import run_bass_kernel_spmd

BF16 = mybir.dt.bfloat16
F32 = mybir.dt.float32
NPBF16 = ml_dtypes.bfloat16

B, S, E = 4, 2048, 1024
D = 64             # head dim
OL = 512           # local output dim (8 heads)
NP = 4             # head pairs per core
N_CORES = 8
QB = 512           # q-block
NQB = 4
NKT = 16           # key tiles of 128
NET = 8            # embed tiles of 128
NSTEP = 256        # NP * NQB * NKT
NWIN = 128


def build_program():
    from contextlib import ExitStack

    nc = bacc.Bacc("TRN2", debug=False, num_devices=N_CORES)

    xp = nc.dram_tensor("xp", [128, 4 * 4096], BF16, kind="ExternalInput")
    wqp = nc.dram_tensor("wqp", [128, 4096], BF16, kind="ExternalInput")
    wkp = nc.dram_tensor("wkp", [128, 4096], BF16, kind="ExternalInput")
    wvp = nc.dram_tensor("wvp", [128, 4096], BF16, kind="ExternalInput")
    wop = nc.dram_tensor("wop", [128, 4096], BF16, kind="ExternalInput")
    bqc = nc.dram_tensor("bqc", [128, 4], F32, kind="ExternalInput")
    yT = nc.dram_tensor("yT", [E, S], F32, kind="ExternalOutput")

    with tile.TileContext(nc) as tc, ExitStack() as est:
        xsb_p = est.enter_context(tc.tile_pool(name="xsb", bufs=4))
        wq_p = est.enter_context(tc.tile_pool(name="wq", bufs=4))
        wk_p = est.enter_context(tc.tile_pool(name="wk", bufs=4))
        wv_p = est.enter_context(tc.tile_pool(name="wv", bufs=1))
        wo_p = est.enter_context(tc.tile_pool(name="wo", bufs=4))
        bias_p = est.enter_context(tc.tile_pool(name="bias", bufs=4))
        qt_p = est.enter_context(tc.tile_pool(name="qt", bufs=4))
        kt_p = est.enter_context(tc.tile_pool(name="kt", bufs=4))
        vb_p = est.enter_context(tc.tile_pool(name="vb", bufs=NKT))
        at_p = est.enter_context(tc.tile_pool(name="at", bufs=16))
        et_p = est.enter_context(tc.tile_pool(name="et", bufs=4))
        usb_p = est.enter_context(tc.tile_pool(name="usb", bufs=2))
        nrm_p = est.enter_context(tc.tile_pool(name="nrm", bufs=4))
        ys_p = est.enter_context(tc.tile_pool(name="ys", bufs=4))
        sc_p = est.enter_context(tc.tile_pool(name="sc", bufs=2,
                                              space="PSUM"))
        u_p = est.enter_context(tc.tile_pool(name="u", bufs=1, space="PSUM"))
        se_psum_p = est.enter_context(tc.tile_pool(name="sep", bufs=1,
                                                   space="PSUM"))
        pj_p = est.enter_context(tc.tile_pool(name="pj", bufs=2, space="PSUM"))

        # ---- ACT table warm-up while DMAs start ----
        warm = bias_p.tile([1, 16], F32, tag="warm", name="warm")
        nc.vector.memset(warm[:], 0.0)
        warm2 = bias_p.tile([1, 16], F32, tag="warm2", name="warm2")
        nc.scalar.activation(warm2[:], warm[:],
                             mybir.ActivationFunctionType.Exp)

        # ---- persistent SBUF tiles ----
        xsb = [xsb_p.tile([128, 4096], BF16, tag="xsb", name="xsb")
               for _ in range(4)]
        wqsb = [wq_p.tile([128, 1024], BF16, tag="wq", name="wqsb")
                for _ in range(4)]
        wksb = [wk_p.tile([128, 1024], BF16, tag="wk", name="wksb")
                for _ in range(4)]
        wvsb = wv_p.tile([128, 4096], BF16, tag="wv", name="wvsb")
        wosb = [wo_p.tile([128, 1024], BF16, tag="wo", name="wosb")
                for _ in range(4)]
        bqs = bias_p.tile([128, 4], F32, tag="bqc", name="bqs")
        ones32 = bias_p.tile([128, 32], BF16, tag="ones32", name="ones32")
        nc.vector.memset(ones32[:], 1.0)

        qts = [qt_p.tile([128, S], BF16, tag="qt", name="qts")
               for _ in range(NP)]
        kts = [kt_p.tile([128, S], BF16, tag="kt", name="kts")
               for _ in range(NP)]
        vbs = [vb_p.tile([128, OL], BF16, tag="vb", name="vbs")
               for _ in range(NKT)]
        atts = [[at_p.tile([128, QB], BF16, tag="at", name="atts")
                 for _ in range(NP)] for _ in range(NQB)]

        # ---- input DMAs, priority order ----
        nc.sync.dma_start(xsb[0][:], xp[:, 0:4096])
        nc.sync.dma_start(wqsb[0][:], wqp[:, 0:1024])
        nc.sync.dma_start(wksb[0][:], wkp[:, 0:1024])
        nc.sync.dma_start(bqs[:], bqc[:])
        nc.sync.dma_start(wvsb[:], wvp[:])
        for j in range(1, 4):
            nc.sync.dma_start(xsb[j][:], xp[:, j * 4096:(j + 1) * 4096])
        for o in range(1, 4):
            nc.sync.dma_start(wksb[o][:], wkp[:, o * 1024:(o + 1) * 1024])
            nc.sync.dma_start(wqsb[o][:], wqp[:, o * 1024:(o + 1) * 1024])
        for p2 in range(4):
            nc.sync.dma_start(wosb[p2][:], wop[:, p2 * 1024:(p2 + 1) * 1024])

        # ---- PSUM scores ping-pong: per-step [128,1024] tiles (2 banks
        # each) from a 2-buf pool; tracked per-tile so exp(s) never
        # serializes against scores(s+1) in the other slot.
        scs = {}   # step -> pending scores tile

        # ---- emitters ----
        def qk_group(p, j, which):
            w = wqsb[p] if which == "q" else wksb[p]
            dest = qts[p] if which == "q" else kts[p]
            acc = pj_p.tile([128, QB], F32, tag="pj", name="pj")
            for e in range(NET):
                nc.tensor.matmul(
                    acc[:],
                    w[:, e * 128:(e + 1) * 128],
                    xsb[j][:, e * 512:(e + 1) * 512],
                    start=(e == 0), stop=(e == NET - 1),
                )
            if which == "q":
                nc.vector.tensor_scalar_add(
                    dest[:, j * QB:(j + 1) * QB], acc[:], bqs[:, p:p + 1])
            else:
                nc.vector.tensor_copy(dest[:, j * QB:(j + 1) * QB], acc[:])

        def v_group(t):
            jj, tt = t // 4, t % 4
            acc = pj_p.tile([128, OL], F32, tag="pj", name="pjv")
            for e in range(NET):
                nc.tensor.matmul(
                    acc[:],
                    xsb[jj][:, e * 512 + tt * 128: e * 512 + (tt + 1) * 128],
                    wvsb[:, e * 512:(e + 1) * 512],
                    start=(e == 0), stop=(e == NET - 1),
                )
            nc.vector.tensor_copy(vbs[t][:], acc[:])

        def og_group(qb, eo):
            y = pj_p.tile([128, QB], F32, tag="pj", name="pjo")
            for p2 in range(4):
                nc.tensor.matmul(
                    y[:],
                    wosb[p2][:, eo * 128:(eo + 1) * 128],
                    atts[qb][p2][:],
                    start=(p2 == 0), stop=(p2 == 3),
                )
            ysb = ys_p.tile([128, QB], F32, tag="ys", name="ysb")
            nc.vector.tensor_copy(ysb[:], y[:])
            nc.sync.dma_start(
                yT[eo * 128:(eo + 1) * 128, qb * QB:(qb + 1) * QB], ysb[:])

        def emit_scores(s):
            p, qb, kt = s // 64, (s % 64) // 16, s % 16
            sc = sc_p.tile([128, 1024], F32, tag="sc", name="sc")
            scs[s] = sc
            for h2 in range(2):
                nc.tensor.matmul(
                    sc[:, h2 * QB:(h2 + 1) * QB],
                    kts[p][h2 * 64:(h2 + 1) * 64, kt * 128:(kt + 1) * 128],
                    qts[p][h2 * 64:(h2 + 1) * 64, qb * QB:(qb + 1) * QB],
                    start=True, stop=True, tile_position=(h2 * 64, 0),
                    skip_group_check=True,
                )

        # ---- filler schedule (window -> list of thunks) ----
        sched = {}

        def put(w, th):
            sched.setdefault(w, []).append(th)

        def qg(p, j):
            return lambda: qk_group(p, j, "q")

        def kg(p, j):
            return lambda: qk_group(p, j, "k")

        for t in range(4, 16):               # V(t) needed at step t
            put((t - 4) // 2, lambda t=t: v_group(t))
        put(0, kg(0, 1))
        put(2, kg(0, 2))
        put(4, kg(0, 3))
        put(6, qg(0, 1))
        put(10, qg(0, 2))
        put(14, qg(0, 3))
        for p in range(1, NP):
            base = {1: 16, 2: 40, 3: 66}[p]
            gap = {1: 2, 2: 3, 3: 4}[p]
            items = [kg(p, 0), kg(p, 1), kg(p, 2), kg(p, 3),
                     qg(p, 0), qg(p, 1), qg(p, 2), qg(p, 3)]
            for idx, th in enumerate(items):
                put(base + gap * idx, th)
        for qb in range(3):                  # og(qb) after pair3 block qb
            for eo in range(NET):
                put(104 + 8 * qb + eo, lambda qb=qb, eo=eo: og_group(qb, eo))

        # ---- prologue ----
        qk_group(0, 0, "q")
        qk_group(0, 0, "k")
        for t in range(4):
            v_group(t)
        emit_scores(0)
        emit_scores(1)

        # ---- main loop ----
        cur = {}
        for w in range(NWIN):
            s0, s1 = 2 * w, 2 * w + 1
            p, qb = s0 // 64, (s0 % 64) // 16
            kt0, kt1 = s0 % 16, s1 % 16

            # exp for s0, s1 (separate so scores(s0+2) can overlap exp(s1))
            et0 = et_p.tile([128, 1024], BF16, tag="et", name="et0")
            nc.scalar.activation(
                et0[:], scs.pop(s0)[:],
                mybir.ActivationFunctionType.Exp, scale=0.125)
            et1 = et_p.tile([128, 1024], BF16, tag="et", name="et1")
            nc.scalar.activation(
                et1[:], scs.pop(s1)[:],
                mybir.ActivationFunctionType.Exp, scale=0.125)

            if kt0 == 0:
                cur["u"] = u_p.tile([128, QB], F32, tag="u", name="u")
                cur["se"] = se_psum_p.tile([128, QB], F32, tag="se",
                                           name="se")

            # fillers
            for th in sched.get(w, []):
                th()

            # scores lookahead (slot 0; gated on exp(s0) only)
            if s0 + 2 < NSTEP:
                emit_scores(s0 + 2)

            # U matmuls for s0, s1 (chained, same accumulation group)
            u = cur["u"]
            for et, kt in ((et0, kt0), (et1, kt1)):
                for h2 in range(2):
                    hl = 2 * p + h2
                    nc.tensor.matmul(
                        u[h2 * 64:(h2 + 1) * 64, :],
                        vbs[kt][:, hl * D:(hl + 1) * D],
                        et[:, h2 * QB:(h2 + 1) * QB],
                        start=(kt == 0), stop=(kt == NKT - 1),
                        tile_position=(0, h2 * 64),
                        skip_group_check=True,
                    )

            # sumexp: 4 col-tiled M=32 matmuls accumulating over the block
            se = cur["se"]
            for g in range(4):
                q2, h2 = g // 2, g % 2
                nc.tensor.matmul(
                    se[g * 32:(g + 1) * 32, :],
                    ones32[:],
                    (et0 if q2 == 0 else et1)[:, h2 * QB:(h2 + 1) * QB],
                    start=(kt0 == 0), stop=(kt1 == NKT - 1),
                    tile_position=(0, g * 32),
                    skip_group_check=True,
                )

            # scores lookahead (slot 1; gated on exp(s1))
            if s0 + 3 < NSTEP:
                emit_scores(s0 + 3)

            # block end: evacuate U, normalize into atts
            if kt1 == NKT - 1:
                ub = usb_p.tile([128, QB], BF16, tag="usb", name="ub")
                nc.vector.tensor_copy(ub[:], u[:])
                for h2 in range(2):
                    sa = nrm_p.tile([1, QB], F32, tag="sa", name="sa")
                    nc.vector.tensor_copy(
                        sa[:], se[h2 * 32:h2 * 32 + 1, :])
                    sb2 = nrm_p.tile([1, QB], F32, tag="sb2", name="sb2")
                    nc.vector.tensor_copy(
                        sb2[:], se[64 + h2 * 32:64 + h2 * 32 + 1, :])
                    rcs = nrm_p.tile([1, QB], F32, tag="rcs", name="rcs")
                    nc.vector.tensor_add(rcs[:], sa[:], sb2[:])
                    rcr = nrm_p.tile([1, QB], F32, tag="rcr", name="rcr")
                    nc.vector.reciprocal_approx_fast(rcr[:], rcs[:])
                    bcf = nrm_p.tile([128, QB], F32, tag="bcf", name="bcf")
                    nc.gpsimd.partition_broadcast(bcf[:], rcr[:])
                    nc.vector.tensor_mul(
                        atts[qb][p][h2 * 64:(h2 + 1) * 64, :],
                        ub[h2 * 64:(h2 + 1) * 64, :],
                        bcf[h2 * 64:(h2 + 1) * 64, :])

        # ---- tail: last q-block's output projection ----
        for eo in range(NET):
            og_group(3, eo)

    nc.compile()
    return nc


_CACHED = {}


def _get_program():
    if "nc" not in _CACHED:
        _CACHED["nc"] = build_program()
    return _CACHED["nc"]


def make_inputs(embeddings, wq, bq, wk, bk, wv, bv, wo, bo):
    """Host-side sharding: per-core input maps."""
    in_maps = []
    for c in range(N_CORES):
        b, half = c // 2, c % 2
        sl = slice(half * OL, (half + 1) * OL)
        # xp[p, j*4096 + e*512 + u] = embeddings[b, j*512+u, e*128+p]
        xpk = np.ascontiguousarray(
            embeddings[b].reshape(4, 512, 8, 128).transpose(3, 0, 2, 1)
            .reshape(128, 16384)).astype(NPBF16)
        # wqp[p, o*1024 + e*128 + c2] = wq[sl][o*128+c2, e*128+p]
        wqk = np.ascontiguousarray(
            wq[sl, :].reshape(4, 128, 8, 128).transpose(3, 0, 2, 1)
            .reshape(128, 4096)).astype(NPBF16)
        wkk = np.ascontiguousarray(
            wk[sl, :].reshape(4, 128, 8, 128).transpose(3, 0, 2, 1)
            .reshape(128, 4096)).astype(NPBF16)
        # wvp[p, e*512 + f] = wv[sl][f, e*128+p]
        wvk = np.ascontiguousarray(
            wv[sl, :].reshape(512, 8, 128).transpose(2, 1, 0)
            .reshape(128, 4096)).astype(NPBF16)
        # wop[p, p2*1024 + c2] = wo[c2, sl][p2*128+p]
        wok = np.ascontiguousarray(
            wo[:, sl].T.reshape(4, 128, 1024).transpose(1, 0, 2)
            .reshape(128, 4096)).astype(NPBF16)
        in_maps.append({
            "xp": xpk,
            "wqp": wqk,
            "wkp": wkk,
            "wvp": wvk,
            "wop": wok,
            "bqc": np.ascontiguousarray(
                bq[sl].reshape(4, 128).T).astype(np.float32),
        })
    return in_maps


def unshard(results, wo, bv, bo):
    bo_eff = bo + wo @ bv
    out = np.empty((B, S, E), np.float32)
    for b in range(B):
        yt = results[2 * b]["yT"] + results[2 * b + 1]["yT"]
        out[b] = yt.T + bo_eff[None, :]
    return out


def kernel(embeddings, wq, bq, wk, bk, wv, bv, wo, bo, _trace=False):
    embeddings = np.asarray(embeddings, np.float32)
    wq, bq = np.asarray(wq, np.float32), np.asarray(bq, np.float32)
    wk, bk = np.asarray(wk, np.float32), np.asarray(bk, np.float32)
    wv, bv = np.asarray(wv, np.float32), np.asarray(bv, np.float32)
    wo, bo = np.asarray(wo, np.float32), np.asarray(bo, np.float32)
    nc = _get_program()
    in_maps = make_inputs(embeddings, wq, bq, wk, bk, wv, bv, wo, bo)
    res = run_bass_kernel_spmd(
        nc, in_maps, core_ids=list(range(N_CORES)), trace=_trace)
    out = unshard(res.results, wo, bv, bo)
    if _trace:
        kernel.last_result = res
    return out
